# revision 20
# baseline (speedup 1.0000x reference)
"""Segment-mean GNN aggregation (MeanAggregator) on 8 TRN2 NeuronCores.

out[v] = mean over edges (u -> v) of x[u], zeros for isolated nodes.

Strategy: shard destination nodes across the 8 cores (12500 each) and
replicate x (stored fp16) in every core's DRAM. The kernel is bound by
the SWDGE dma_gather rate (~2.5 ns/desc aggregate across 4 SWDGE
queues; each queue drains at ~30 GB/s for 256 B descriptors), so the
design keeps all 4 queues continuously fed and moves everything else
off the critical path:

- Each core's edges are bucketed into (segment, bank) cells, where a
  segment is 4 consecutive 128-dst groups and a bank is a 25000-row
  src window (dma_gather's int16 reach). Cell streams are sorted by
  (group, src) and sized to the exact max across cores (SPMD: one
  program, per-core data). Pads fetch row 0; their S entries are 0.
- Each bank's stream is gathered by uniform 2048-desc ops (after a
  512/1536 ramp that gets all 4 queues engaged within ~1.4 us) emitted
  round-robin across banks, one SWDGE queue per bank, with a hoisted
  num_idxs register (one MOVE per distinct size). The first round of
  index tiles is DMA-loaded by GpSimd itself (same-engine dependency);
  the rest are loaded up front on the Sync engine so gathers never
  wait behind consumer-side DMAs (head-of-line blocking).
- A "chunk" is a static 128-slot window of a bank stream. The host
  computes the union over cores of (group, bank, chunk) incidences and
  the program runs one matmul per such pair. The per-core S matrix
  (slot -> one-hot dst-in-group) is built on the host in fp8e4m3
  (exact for 0/1/2) and streamed from DRAM on the Sync engine; the
  fp8 lhsT x fp16 rhs mixed matmul is exact for these values.
- ScalarE (Act) copies PSUM scaled by 1/max(deg,1) into 8-group tiles
  and issues the output stores, keeping the consumer-side DMA queue
  separate from the producer-side one.
"""

import math
from contextlib import ExitStack

import ml_dtypes
import numpy as np

import concourse.tile as tile
from concourse import bacc, mybir
from concourse.bass_utils import run_bass_kernel_spmd

N_NODES = 100000
N_FEAT = 128
N_CORES = 8
NODES_PER_CORE = N_NODES // N_CORES  # 12500
P = 128
N_GROUPS = math.ceil(NODES_PER_CORE / P)  # 98
SEG = 4  # groups per segment cell
N_SEGS = math.ceil(N_GROUPS / SEG)  # 25
N_BANKS = 4
BANK = N_NODES // N_BANKS  # 25000
OP_IDX = 2048  # descriptors per gather op
SBATCH = 32  # pairs per streamed S tile
PF_GROUPS = 12  # gather prefetch horizon (groups)
STORE_GROUPS = 8

_compiled_cache = {}


def _plan(cell_len, pair_key):
    """Static structure. cell_len: (N_SEGS, N_BANKS) stream lengths
    (exact max over cores). pair_key: tuple of (g, b, chunk) pairs."""
    cell_len = np.asarray(cell_len)
    # bank stream length: multiple of 16 for the idx-table wrap
    bank_len = [int(-16 * (-cell_len[:, b].sum() // 16))
                for b in range(N_BANKS)]
    cell_off = np.zeros((N_SEGS, N_BANKS), np.int64)
    for b in range(N_BANKS):
        off = 0
        for s in range(N_SEGS):
            cell_off[s, b] = off
            off += int(cell_len[s, b])
    # uniform ops of OP_IDX descs (last partial, 16-granular)
    bank_ops = []
    for b in range(N_BANKS):
        ops = []
        done = 0
        ramp = [512, 1536]  # stagger-killer: all 4 queues engage within ~1.4us
        while done < bank_len[b]:
            want = ramp.pop(0) if ramp else OP_IDX
            n = min(want, bank_len[b] - done)
            ops.append((done, n))
            done += n
        bank_ops.append(ops)
    pairs = [tuple(p) for p in pair_key]
    return {
        "cell_len": cell_len,
        "cell_off": cell_off,
        "bank_len": bank_len,
        "bank_ops": bank_ops,
        "pairs": pairs,
    }


def _build_kernel(key):
    cell_len_key, pair_key = key
    plan = _plan(np.asarray(cell_len_key).reshape(N_SEGS, N_BANKS), pair_key)
    bank_len = plan["bank_len"]
    bank_ops = plan["bank_ops"]
    cell_off = plan["cell_off"]
    cell_len = plan["cell_len"]
    pairs = plan["pairs"]
    n_pairs = len(pairs)
    n_batches = math.ceil(n_pairs / SBATCH)

    nc = bacc.Bacc("TRN2", target_bir_lowering=False, debug=False,
                   num_devices=N_CORES, num_swdge_queues=N_BANKS)
    f16, f32, f8 = mybir.dt.float16, mybir.dt.float32, mybir.dt.float8e4
    x_d = nc.dram_tensor("x", [N_NODES, N_FEAT], f16,
                         kind="ExternalInput").ap()
    idx_ds = [nc.dram_tensor(f"midx{b}", [P, bank_len[b] // 16],
                             mybir.dt.int16, kind="ExternalInput").ap()
              for b in range(N_BANKS)]
    s_d = nc.dram_tensor("msel", [P, n_pairs * P], f8,
                         kind="ExternalInput").ap()
    invd_d = nc.dram_tensor("minvd", [P, N_GROUPS], f32,
                            kind="ExternalInput").ap()
    out_d = nc.dram_tensor("out", [NODES_PER_CORE, N_FEAT], f32,
                           kind="ExternalOutput").ap()

    n_ops_total = sum(len(o) for o in bank_ops)

    with tile.TileContext(nc) as tc, ExitStack() as ctx:
        meta_pool = ctx.enter_context(tc.tile_pool(name="meta", bufs=1))
        idx_pool = ctx.enter_context(
            tc.tile_pool(name="idx", bufs=n_ops_total))
        gat_pool = ctx.enter_context(tc.tile_pool(name="gat", bufs=22))
        sel_pool = ctx.enter_context(tc.tile_pool(name="sel", bufs=10))
        psum_pool = ctx.enter_context(
            tc.tile_pool(name="psum", bufs=8, space="PSUM"))
        out_pool = ctx.enter_context(tc.tile_pool(name="outb", bufs=3))

        # idx tiles up front; first round split across both HWDGE engines
        # so the first gathers start as early as possible, then the first
        # two S tiles, then the bulk (Sync only; nothing can block these).
        idx_tiles = {}

        def load_idx(b, oi, eng):
            off, n = bank_ops[b][oi]
            i_t = idx_pool.tile([P, OP_IDX // 16], mybir.dt.int16, tag="idx")
            eng.dma_start(out=i_t[:, :n // 16],
                          in_=idx_ds[b][:, off // 16:(off + n) // 16])
            idx_tiles[(b, oi)] = i_t

        # warm-up: a 16-desc gather per queue from a device-MEMSET idx tile
        # (no DMA dependency) pays the cold SWDGE/ucode start concurrently
        # with the first real idx-table DMA latency
        warm_reg = nc.gpsimd.to_reg(16)
        for b in range(N_BANKS):
            w_i = meta_pool.tile([P, 1], mybir.dt.int16, tag=f"warmi{b}")
            nc.gpsimd.memset(w_i[:], 0)
            w_g = meta_pool.tile([P, 1, N_FEAT], f16, tag=f"warmg{b}")
            nc.gpsimd.dma_gather(
                out_ap=w_g[:, :, :],
                in_ap=x_d[b * BANK:(b + 1) * BANK, :],
                idxs_ap=w_i[:, :],
                num_idxs=16,
                num_idxs_reg=warm_reg,
                elem_size=N_FEAT,
                queue_num=b,
                single_packet=False,
            )

        # first round on GpSimd itself: same-engine dependency lets the
        # first gathers start ~9 us earlier than a cross-engine sem wait
        for b in range(N_BANKS):
            load_idx(b, 0, nc.gpsimd)

        emitted_ops = [0] * N_BANKS
        op_tiles = {}  # (b, op_i) -> gather tile
        # chunk -> (op index, column within op tile) per bank
        chunk_op = []
        for b in range(N_BANKS):
            m = {}
            for oi, (off, n) in enumerate(bank_ops[b]):
                for j in range(-(-n // 128)):
                    m[off // 128 + j] = (oi, j)
            chunk_op.append(m)

        nreg = {}

        def emit_one_op(b):
            oi = emitted_ops[b]
            off, n = bank_ops[b][oi]
            i_t = idx_tiles[(b, oi)]
            g_t = gat_pool.tile([P, OP_IDX // 128, N_FEAT], f16, tag="gat")
            if n not in nreg:
                nreg[n] = nc.gpsimd.to_reg(n)
            nc.gpsimd.dma_gather(
                out_ap=g_t[:, :-(-n // 128), :],
                in_ap=x_d[b * BANK:(b + 1) * BANK, :],
                idxs_ap=i_t[:, :n // 16],
                num_idxs=n,
                num_idxs_reg=nreg[n],
                elem_size=N_FEAT,
                queue_num=b,
                single_packet=False,
            )
            op_tiles[(b, oi)] = g_t
            emitted_ops[b] += 1

        def emit_until_group(g):
            """Round-robin emission until every bank covers group g's cells."""
            s = min(g // SEG, N_SEGS - 1)
            need_op = [0] * N_BANKS
            for b in range(N_BANKS):
                end = int(cell_off[s, b] + cell_len[s, b])
                if end == 0:
                    need_op[b] = -1
                else:
                    need_op[b] = chunk_op[b][(end - 1) // 128][0]
            progress = True
            while progress:
                progress = False
                for b in range(N_BANKS):
                    if emitted_ops[b] <= need_op[b] and \
                            emitted_ops[b] < len(bank_ops[b]):
                        emit_one_op(b)
                        progress = True

        s_tiles = {}

        def emit_sbatch(bi):
            if bi in s_tiles or bi >= n_batches:
                return
            c0 = bi * SBATCH
            n = min(SBATCH, n_pairs - c0)
            s_t = sel_pool.tile([P, SBATCH * P], f8, tag="sel")
            nc.sync.dma_start(out=s_t[:, :n * P],
                              in_=s_d[:, c0 * P:(c0 + n) * P])
            s_tiles[bi] = s_t

        # group -> list of pair indices (in emission order)
        group_pairs = {}
        for pi, (g, b, c) in enumerate(pairs):
            group_pairs.setdefault(g, []).append(pi)

        # prime the pipeline: first S tiles early on the Sync queue, then
        # the remaining idx tiles, invd, and the gather run-ahead.
        emit_sbatch(0)
        emit_sbatch(1)
        for oi in range(1, max(len(o) for o in bank_ops)):
            for b in range(N_BANKS):
                if oi < len(bank_ops[b]):
                    load_idx(b, oi, nc.sync)
        invd_t = meta_pool.tile([P, N_GROUPS], f32)
        nc.scalar.dma_start(out=invd_t[:], in_=invd_d[:])
        emit_until_group(min(PF_GROUPS, N_GROUPS - 1))

        out_t = None
        for g in range(N_GROUPS):
            if g + PF_GROUPS < N_GROUPS:
                emit_until_group(g + PF_GROUPS)
            plist = group_pairs[g]
            ps = psum_pool.tile([P, N_FEAT], f32)
            for i, pi in enumerate(plist):
                _, b, c = pairs[pi]
                bi = pi // SBATCH
                emit_sbatch(bi)
                emit_sbatch(bi + 1)
                s_t = s_tiles[bi]
                lc = pi - bi * SBATCH
                oi, col = chunk_op[b][c]
                g_t = op_tiles[(b, oi)]
                nc.tensor.matmul(
                    ps[:],
                    lhsT=s_t[:, lc * P:(lc + 1) * P],
                    rhs=g_t[:, col, :],
                    start=(i == 0),
                    stop=(i == len(plist) - 1),
                )
            if g % STORE_GROUPS == 0:
                out_t = out_pool.tile([P, STORE_GROUPS, N_FEAT], f32,
                                      tag="outb")
            nc.scalar.activation(out=out_t[:, g % STORE_GROUPS, :], in_=ps[:],
                                 func=mybir.ActivationFunctionType.Copy,
                                 scale=invd_t[:, g:g + 1])
            if g % STORE_GROUPS == STORE_GROUPS - 1 or g == N_GROUPS - 1:
                g0 = (g // STORE_GROUPS) * STORE_GROUPS
                ngroups = g - g0 + 1
                nfull = ngroups
                rows_last = min(P, NODES_PER_CORE - (g0 + ngroups - 1) * P)
                if rows_last < P:
                    nfull -= 1
                if nfull > 0:
                    dst = out_d[g0 * P:(g0 + nfull) * P, :].rearrange(
                        "(j p) f -> p j f", p=P)
                    nc.scalar.dma_start(out=dst, in_=out_t[:, :nfull, :])
                if nfull < ngroups:
                    gl = g0 + ngroups - 1
                    nc.scalar.dma_start(
                        out=out_d[gl * P:gl * P + rows_last, :],
                        in_=out_t[:rows_last, ngroups - 1, :])
    nc.compile()
    return nc


def _prepare(x, edge_src, edge_dst):
    x16 = np.ascontiguousarray(np.asarray(x), dtype=np.float16)
    src = np.asarray(edge_src).astype(np.int64)
    dst = np.asarray(edge_dst).astype(np.int64)

    deg = np.bincount(dst, minlength=N_NODES)
    inv_deg = (1.0 / np.maximum(deg, 1)).astype(np.float32)

    core_e = dst // NODES_PER_CORE
    ldst = dst % NODES_PER_CORE
    g_e = ldst // P
    s_e = g_e // SEG
    b_e = src // BANK

    cnt = np.zeros((N_CORES, N_SEGS, N_BANKS), np.int64)
    np.add.at(cnt, (core_e, s_e, b_e), 1)
    cell_len = cnt.max(axis=0).astype(np.int64)

    plan = _plan(cell_len, ())
    cell_off = plan["cell_off"]
    bank_len = plan["bank_len"]

    # per-core packing: stream position of every edge
    per_core = []
    pair_set = [set() for _ in range(N_GROUPS)]
    for k in range(N_CORES):
        m = core_e == k
        ksrc, kg, kb, ks = src[m], g_e[m], b_e[m], s_e[m]
        kldst = ldst[m]
        order = np.lexsort((ksrc, kg, kb, ks))
        ksrc, kg, kb, ks, kldst = (ksrc[order], kg[order], kb[order],
                                   ks[order], kldst[order])
        cid = ks * N_BANKS + kb
        pos = np.zeros(len(ksrc), np.int64)
        uniq, starts, counts = np.unique(cid, return_index=True,
                                         return_counts=True)
        for u, st, n in zip(uniq, starts, counts):
            s, b = int(u) // N_BANKS, int(u) % N_BANKS
            assert n <= cell_len[s, b]
            pos[st:st + n] = cell_off[s, b] + np.arange(n)
        chunk = pos // 128
        for g in range(N_GROUPS):
            gm = kg == g
            if not gm.any():
                continue
            for b, c in set(zip(kb[gm].tolist(), chunk[gm].tolist())):
                pair_set[g].add((b, c))
        per_core.append((ksrc, kg, kb, kldst, pos))

    # pair list in emission order (group-major, then bank, then chunk)
    pairs = []
    pair_index = {}
    for g in range(N_GROUPS):
        cells = sorted(pair_set[g])
        if not cells:
            cells = [(0, 0)]  # dummy pair so psum gets written (S is 0)
        for b, c in cells:
            pair_index[(g, b, c)] = len(pairs)
            pairs.append((g, b, c))
    pair_key = tuple(pairs)
    n_pairs = len(pairs)

    in_maps = []
    for k in range(N_CORES):
        ksrc, kg, kb, kldst, pos = per_core[k]
        idxs = {}
        for b in range(N_BANKS):
            st = np.zeros(bank_len[b], np.int16)
            mb = kb == b
            st[pos[mb]] = (ksrc[mb] - b * BANK).astype(np.int16)
            # wrap-16 layout replicated to 128 partitions
            idxs[f"midx{b}"] = np.ascontiguousarray(
                np.tile(st.reshape(-1, 16).T, (8, 1)))
        # host-built S: [slot-in-chunk, pair*128 + dst-in-group] one-hot
        s_tab = np.zeros((P, n_pairs * P), np.float32)
        pidx = np.fromiter(
            (pair_index[(g, b, c)] for g, b, c in
             zip(kg.tolist(), kb.tolist(), (pos // 128).tolist())),
            np.int64, len(kg))
        dd = (kldst - kg * P).astype(np.int64)
        np.add.at(s_tab, (pos % 128, pidx * P + dd), 1.0)
        invd = np.zeros((N_GROUPS * P,), np.float32)
        invd[:NODES_PER_CORE] = inv_deg[k * NODES_PER_CORE:
                                        (k + 1) * NODES_PER_CORE]
        in_maps.append({
            "x": x16,
            **idxs,
            "msel": s_tab.astype(ml_dtypes.float8_e4m3),
            "minvd": np.ascontiguousarray(invd.reshape(N_GROUPS, P).T),
        })
    key = (tuple(int(v) for v in cell_len.ravel()), pair_key)
    kernel.last_stats = {"total_len": int(sum(bank_len)), "n_pairs": n_pairs}
    return in_maps, key


def kernel(x, edge_src, edge_dst, _trace=False):
    in_maps, key = _prepare(x, edge_src, edge_dst)
    nc = _compiled_cache.get(key)
    if nc is None:
        nc = _build_kernel(key)
        _compiled_cache[key] = nc
    res = run_bass_kernel_spmd(nc, in_maps, core_ids=list(range(N_CORES)),
                               trace=_trace)
    out = np.concatenate([res.results[k]["out"] for k in range(N_CORES)],
                         axis=0)
    if _trace:
        kernel.last_exec_time_ns = res.exec_time_ns
        kernel.last_result = res
    return out


# revision 21
# speedup vs baseline: 1.0186x; 1.0186x over previous
"""Segment-mean GNN aggregation (MeanAggregator) on 8 TRN2 NeuronCores.

out[v] = mean over edges (u -> v) of x[u], zeros for isolated nodes.

Strategy: shard destination nodes across the 8 cores (12500 each) and
replicate x (stored fp16) in every core's DRAM. The kernel is bound by
the SWDGE dma_gather rate (~2.5 ns/desc aggregate across 4 SWDGE
queues; each queue drains at ~30 GB/s for 256 B descriptors), so the
design keeps all 4 queues continuously fed and moves everything else
off the critical path:

- Each core's edges are bucketed into (segment, bank) cells, where a
  segment is 4 consecutive 128-dst groups and a bank is a 25000-row
  src window (dma_gather's int16 reach). Cell streams are sorted by
  (group, src) and sized to the exact max across cores (SPMD: one
  program, per-core data). Pads fetch row 0; their S entries are 0.
- Each bank's stream is gathered by uniform 2048-desc ops (after a
  512/1536 ramp that gets all 4 queues engaged within ~1.4 us) emitted
  round-robin across banks, one SWDGE queue per bank, with a hoisted
  num_idxs register (one MOVE per distinct size). The first round of
  index tiles is DMA-loaded by GpSimd itself (same-engine dependency);
  the rest are loaded up front on the Sync engine so gathers never
  wait behind consumer-side DMAs (head-of-line blocking).
- A "chunk" is a static 128-slot window of a bank stream. The host
  computes the union over cores of (group, bank, chunk) incidences and
  the program runs one matmul per such pair. The per-core S matrix
  (slot -> one-hot dst-in-group) is built on the host in fp8e4m3
  (exact for 0/1/2) and streamed from DRAM on the Sync engine; the
  fp8 lhsT x fp16 rhs mixed matmul is exact for these values.
- ScalarE (Act) copies PSUM scaled by 1/max(deg,1) into 8-group tiles
  and issues the output stores, keeping the consumer-side DMA queue
  separate from the producer-side one.
"""

import math
from contextlib import ExitStack

import ml_dtypes
import numpy as np

import concourse.tile as tile
from concourse import bacc, mybir
from concourse.bass_utils import run_bass_kernel_spmd

N_NODES = 100000
N_FEAT = 128
N_CORES = 8
NODES_PER_CORE = N_NODES // N_CORES  # 12500
P = 128
N_GROUPS = math.ceil(NODES_PER_CORE / P)  # 98
SEG = 4  # groups per segment cell
N_SEGS = math.ceil(N_GROUPS / SEG)  # 25
N_BANKS = 4
BANK = N_NODES // N_BANKS  # 25000
OP_IDX = 2048  # descriptors per gather op
SBATCH = 32  # pairs per streamed S tile
PF_GROUPS = 12  # gather prefetch horizon (groups)
STORE_GROUPS = 8

_compiled_cache = {}


def _plan(cell_len, pair_key):
    """Static structure. cell_len: (N_SEGS, N_BANKS) stream lengths
    (exact max over cores). pair_key: tuple of (g, b, chunk) pairs."""
    cell_len = np.asarray(cell_len)
    # bank stream length: multiple of 16 for the idx-table wrap
    bank_len = [int(-16 * (-cell_len[:, b].sum() // 16))
                for b in range(N_BANKS)]
    cell_off = np.zeros((N_SEGS, N_BANKS), np.int64)
    for b in range(N_BANKS):
        off = 0
        for s in range(N_SEGS):
            cell_off[s, b] = off
            off += int(cell_len[s, b])
    # uniform ops of OP_IDX descs (last partial, 16-granular)
    bank_ops = []
    for b in range(N_BANKS):
        ops = []
        done = 0
        ramp = [512, 1536]  # stagger-killer: all 4 queues engage within ~1.4us
        while done < bank_len[b]:
            want = ramp.pop(0) if ramp else OP_IDX
            n = min(want, bank_len[b] - done)
            ops.append((done, n))
            done += n
        bank_ops.append(ops)
    pairs = [tuple(p) for p in pair_key]
    return {
        "cell_len": cell_len,
        "cell_off": cell_off,
        "bank_len": bank_len,
        "bank_ops": bank_ops,
        "pairs": pairs,
    }


def _build_kernel(key):
    cell_len_key, pair_key = key
    plan = _plan(np.asarray(cell_len_key).reshape(N_SEGS, N_BANKS), pair_key)
    bank_len = plan["bank_len"]
    bank_ops = plan["bank_ops"]
    cell_off = plan["cell_off"]
    cell_len = plan["cell_len"]
    pairs = plan["pairs"]
    n_pairs = len(pairs)
    n_batches = math.ceil(n_pairs / SBATCH)

    nc = bacc.Bacc("TRN2", target_bir_lowering=False, debug=False,
                   num_devices=N_CORES, num_swdge_queues=N_BANKS)
    f16, f32, f8 = mybir.dt.float16, mybir.dt.float32, mybir.dt.float8e4
    x_d = nc.dram_tensor("x", [N_NODES, N_FEAT], f16,
                         kind="ExternalInput").ap()
    idx_ds = [nc.dram_tensor(f"midx{b}", [P, bank_len[b] // 16],
                             mybir.dt.int16, kind="ExternalInput").ap()
              for b in range(N_BANKS)]
    s_d = nc.dram_tensor("msel", [P, n_pairs * P], f8,
                         kind="ExternalInput").ap()
    invd_d = nc.dram_tensor("minvd", [P, N_GROUPS], f32,
                            kind="ExternalInput").ap()
    out_d = nc.dram_tensor("out", [NODES_PER_CORE, N_FEAT], f32,
                           kind="ExternalOutput").ap()

    n_ops_total = sum(len(o) for o in bank_ops)

    with tile.TileContext(nc) as tc, ExitStack() as ctx:
        meta_pool = ctx.enter_context(tc.tile_pool(name="meta", bufs=1))
        idx_pool = ctx.enter_context(
            tc.tile_pool(name="idx", bufs=n_ops_total))
        gat_pool = ctx.enter_context(tc.tile_pool(name="gat", bufs=22))
        sel_pool = ctx.enter_context(tc.tile_pool(name="sel", bufs=10))
        psum_pool = ctx.enter_context(
            tc.tile_pool(name="psum", bufs=8, space="PSUM"))
        out_pool = ctx.enter_context(tc.tile_pool(name="outb", bufs=3))

        # idx tiles up front; first round split across both HWDGE engines
        # so the first gathers start as early as possible, then the first
        # two S tiles, then the bulk (Sync only; nothing can block these).
        idx_tiles = {}

        def load_idx(b, oi, eng):
            off, n = bank_ops[b][oi]
            i_t = idx_pool.tile([P, OP_IDX // 16], mybir.dt.int16, tag="idx")
            eng.dma_start(out=i_t[:, :n // 16],
                          in_=idx_ds[b][:, off // 16:(off + n) // 16])
            idx_tiles[(b, oi)] = i_t

        # first round on GpSimd itself: same-engine dependency lets the
        # first gathers start ~9 us earlier than a cross-engine sem wait
        for b in range(N_BANKS):
            load_idx(b, 0, nc.gpsimd)

        emitted_ops = [0] * N_BANKS
        op_tiles = {}  # (b, op_i) -> gather tile
        # chunk -> (op index, column within op tile) per bank
        chunk_op = []
        for b in range(N_BANKS):
            m = {}
            for oi, (off, n) in enumerate(bank_ops[b]):
                for j in range(-(-n // 128)):
                    m[off // 128 + j] = (oi, j)
            chunk_op.append(m)

        nreg = {}

        def emit_one_op(b):
            oi = emitted_ops[b]
            off, n = bank_ops[b][oi]
            i_t = idx_tiles[(b, oi)]
            g_t = gat_pool.tile([P, OP_IDX // 128, N_FEAT], f16, tag="gat")
            if n not in nreg:
                nreg[n] = nc.gpsimd.to_reg(n)
            nc.gpsimd.dma_gather(
                out_ap=g_t[:, :-(-n // 128), :],
                in_ap=x_d[b * BANK:(b + 1) * BANK, :],
                idxs_ap=i_t[:, :n // 16],
                num_idxs=n,
                num_idxs_reg=nreg[n],
                elem_size=N_FEAT,
                queue_num=b,
                single_packet=False,
            )
            op_tiles[(b, oi)] = g_t
            emitted_ops[b] += 1

        def emit_until_group(g):
            """Round-robin emission until every bank covers group g's cells."""
            s = min(g // SEG, N_SEGS - 1)
            need_op = [0] * N_BANKS
            for b in range(N_BANKS):
                end = int(cell_off[s, b] + cell_len[s, b])
                if end == 0:
                    need_op[b] = -1
                else:
                    need_op[b] = chunk_op[b][(end - 1) // 128][0]
            progress = True
            while progress:
                progress = False
                for b in range(N_BANKS):
                    if emitted_ops[b] <= need_op[b] and \
                            emitted_ops[b] < len(bank_ops[b]):
                        emit_one_op(b)
                        progress = True

        s_tiles = {}

        def emit_sbatch(bi):
            if bi in s_tiles or bi >= n_batches:
                return
            c0 = bi * SBATCH
            n = min(SBATCH, n_pairs - c0)
            s_t = sel_pool.tile([P, SBATCH * P], f8, tag="sel")
            nc.sync.dma_start(out=s_t[:, :n * P],
                              in_=s_d[:, c0 * P:(c0 + n) * P])
            s_tiles[bi] = s_t

        # group -> list of pair indices (in emission order)
        group_pairs = {}
        for pi, (g, b, c) in enumerate(pairs):
            group_pairs.setdefault(g, []).append(pi)

        # prime the pipeline: first S tiles early on the Sync queue, then
        # the remaining idx tiles, invd, and the gather run-ahead.
        emit_sbatch(0)
        emit_sbatch(1)
        for oi in range(1, max(len(o) for o in bank_ops)):
            for b in range(N_BANKS):
                if oi < len(bank_ops[b]):
                    load_idx(b, oi, nc.sync)
        invd_t = meta_pool.tile([P, N_GROUPS], f32)
        nc.scalar.dma_start(out=invd_t[:], in_=invd_d[:])
        emit_until_group(min(PF_GROUPS, N_GROUPS - 1))

        out_t = None
        for g in range(N_GROUPS):
            if g + PF_GROUPS < N_GROUPS:
                emit_until_group(g + PF_GROUPS)
            plist = group_pairs[g]
            ps = psum_pool.tile([P, N_FEAT], f32)
            for i, pi in enumerate(plist):
                _, b, c = pairs[pi]
                bi = pi // SBATCH
                emit_sbatch(bi)
                emit_sbatch(bi + 1)
                s_t = s_tiles[bi]
                lc = pi - bi * SBATCH
                oi, col = chunk_op[b][c]
                g_t = op_tiles[(b, oi)]
                nc.tensor.matmul(
                    ps[:],
                    lhsT=s_t[:, lc * P:(lc + 1) * P],
                    rhs=g_t[:, col, :],
                    start=(i == 0),
                    stop=(i == len(plist) - 1),
                )
            if g % STORE_GROUPS == 0:
                out_t = out_pool.tile([P, STORE_GROUPS, N_FEAT], f32,
                                      tag="outb")
            nc.scalar.activation(out=out_t[:, g % STORE_GROUPS, :], in_=ps[:],
                                 func=mybir.ActivationFunctionType.Copy,
                                 scale=invd_t[:, g:g + 1])
            if g % STORE_GROUPS == STORE_GROUPS - 1 or g == N_GROUPS - 1:
                g0 = (g // STORE_GROUPS) * STORE_GROUPS
                ngroups = g - g0 + 1
                nfull = ngroups
                rows_last = min(P, NODES_PER_CORE - (g0 + ngroups - 1) * P)
                if rows_last < P:
                    nfull -= 1
                if nfull > 0:
                    dst = out_d[g0 * P:(g0 + nfull) * P, :].rearrange(
                        "(j p) f -> p j f", p=P)
                    nc.scalar.dma_start(out=dst, in_=out_t[:, :nfull, :])
                if nfull < ngroups:
                    gl = g0 + ngroups - 1
                    nc.scalar.dma_start(
                        out=out_d[gl * P:gl * P + rows_last, :],
                        in_=out_t[:rows_last, ngroups - 1, :])
    nc.compile()
    return nc


def _prepare(x, edge_src, edge_dst):
    x16 = np.ascontiguousarray(np.asarray(x), dtype=np.float16)
    src = np.asarray(edge_src).astype(np.int64)
    dst = np.asarray(edge_dst).astype(np.int64)

    deg = np.bincount(dst, minlength=N_NODES)
    inv_deg = (1.0 / np.maximum(deg, 1)).astype(np.float32)

    core_e = dst // NODES_PER_CORE
    ldst = dst % NODES_PER_CORE
    g_e = ldst // P
    s_e = g_e // SEG
    b_e = src // BANK

    cnt = np.zeros((N_CORES, N_SEGS, N_BANKS), np.int64)
    np.add.at(cnt, (core_e, s_e, b_e), 1)
    cell_len = cnt.max(axis=0).astype(np.int64)

    plan = _plan(cell_len, ())
    cell_off = plan["cell_off"]
    bank_len = plan["bank_len"]

    # per-core packing: stream position of every edge
    per_core = []
    pair_set = [set() for _ in range(N_GROUPS)]
    for k in range(N_CORES):
        m = core_e == k
        ksrc, kg, kb, ks = src[m], g_e[m], b_e[m], s_e[m]
        kldst = ldst[m]
        order = np.lexsort((ksrc, kg, kb, ks))
        ksrc, kg, kb, ks, kldst = (ksrc[order], kg[order], kb[order],
                                   ks[order], kldst[order])
        cid = ks * N_BANKS + kb
        pos = np.zeros(len(ksrc), np.int64)
        uniq, starts, counts = np.unique(cid, return_index=True,
                                         return_counts=True)
        for u, st, n in zip(uniq, starts, counts):
            s, b = int(u) // N_BANKS, int(u) % N_BANKS
            assert n <= cell_len[s, b]
            pos[st:st + n] = cell_off[s, b] + np.arange(n)
        chunk = pos // 128
        for g in range(N_GROUPS):
            gm = kg == g
            if not gm.any():
                continue
            for b, c in set(zip(kb[gm].tolist(), chunk[gm].tolist())):
                pair_set[g].add((b, c))
        per_core.append((ksrc, kg, kb, kldst, pos))

    # pair list in emission order (group-major, then bank, then chunk)
    pairs = []
    pair_index = {}
    for g in range(N_GROUPS):
        cells = sorted(pair_set[g])
        if not cells:
            cells = [(0, 0)]  # dummy pair so psum gets written (S is 0)
        for b, c in cells:
            pair_index[(g, b, c)] = len(pairs)
            pairs.append((g, b, c))
    pair_key = tuple(pairs)
    n_pairs = len(pairs)

    in_maps = []
    for k in range(N_CORES):
        ksrc, kg, kb, kldst, pos = per_core[k]
        idxs = {}
        for b in range(N_BANKS):
            st = np.zeros(bank_len[b], np.int16)
            mb = kb == b
            st[pos[mb]] = (ksrc[mb] - b * BANK).astype(np.int16)
            # wrap-16 layout replicated to 128 partitions
            idxs[f"midx{b}"] = np.ascontiguousarray(
                np.tile(st.reshape(-1, 16).T, (8, 1)))
        # host-built S: [slot-in-chunk, pair*128 + dst-in-group] one-hot
        s_tab = np.zeros((P, n_pairs * P), np.float32)
        pidx = np.fromiter(
            (pair_index[(g, b, c)] for g, b, c in
             zip(kg.tolist(), kb.tolist(), (pos // 128).tolist())),
            np.int64, len(kg))
        dd = (kldst - kg * P).astype(np.int64)
        np.add.at(s_tab, (pos % 128, pidx * P + dd), 1.0)
        invd = np.zeros((N_GROUPS * P,), np.float32)
        invd[:NODES_PER_CORE] = inv_deg[k * NODES_PER_CORE:
                                        (k + 1) * NODES_PER_CORE]
        in_maps.append({
            "x": x16,
            **idxs,
            "msel": s_tab.astype(ml_dtypes.float8_e4m3),
            "minvd": np.ascontiguousarray(invd.reshape(N_GROUPS, P).T),
        })
    key = (tuple(int(v) for v in cell_len.ravel()), pair_key)
    kernel.last_stats = {"total_len": int(sum(bank_len)), "n_pairs": n_pairs}
    return in_maps, key


def kernel(x, edge_src, edge_dst, _trace=False):
    in_maps, key = _prepare(x, edge_src, edge_dst)
    nc = _compiled_cache.get(key)
    if nc is None:
        nc = _build_kernel(key)
        _compiled_cache[key] = nc
    res = run_bass_kernel_spmd(nc, in_maps, core_ids=list(range(N_CORES)),
                               trace=_trace)
    out = np.concatenate([res.results[k]["out"] for k in range(N_CORES)],
                         axis=0)
    if _trace:
        kernel.last_exec_time_ns = res.exec_time_ns
        kernel.last_result = res
    return out


# revision 22
# speedup vs baseline: 1.0527x; 1.0334x over previous
"""Segment-mean GNN aggregation (MeanAggregator) on 8 TRN2 NeuronCores.

out[v] = mean over edges (u -> v) of x[u], zeros for isolated nodes.

Strategy: shard destination nodes across the 8 cores (12500 each) and
replicate x (stored fp16) in every core's DRAM. The kernel is bound by
the SWDGE dma_gather rate (~2.5 ns/desc aggregate across 4 SWDGE
queues; each queue drains at ~30 GB/s for 256 B descriptors), so the
design keeps all 4 queues continuously fed and moves everything else
off the critical path:

- Each core's edges are bucketed into (segment, bank) cells, where a
  segment is 4 consecutive 128-dst groups and a bank is a 25000-row
  src window (dma_gather's int16 reach). Cell streams are sorted by
  (group, src) and sized to the exact max across cores (SPMD: one
  program, per-core data). Pads fetch row 0; their S entries are 0.
- Each bank's stream is gathered by uniform 2048-desc ops (after a
  512/1536 ramp that gets all 4 queues engaged within ~1.4 us) emitted
  round-robin across banks, one SWDGE queue per bank, with a hoisted
  num_idxs register (one MOVE per distinct size). The first round of
  index tiles is DMA-loaded by GpSimd itself (same-engine dependency);
  the rest are loaded up front on the Sync engine so gathers never
  wait behind consumer-side DMAs (head-of-line blocking).
- A "chunk" is a static 128-slot window of a bank stream. The host
  computes the union over cores of (group, bank, chunk) incidences and
  the program runs one matmul per such pair. The per-core S matrix
  (slot -> one-hot dst-in-group) is built on the host in fp8e4m3
  (exact for 0/1/2) and streamed from DRAM on the Sync engine; the
  fp8 lhsT x fp16 rhs mixed matmul is exact for these values.
- ScalarE (Act) copies PSUM scaled by 1/max(deg,1) into 8-group tiles
  and issues the output stores, keeping the consumer-side DMA queue
  separate from the producer-side one.
"""

import math
from contextlib import ExitStack

import ml_dtypes
import numpy as np

import concourse.tile as tile
from concourse import bacc, mybir
from concourse.bass_utils import run_bass_kernel_spmd

N_NODES = 100000
N_FEAT = 128
N_CORES = 8
NODES_PER_CORE = N_NODES // N_CORES  # 12500
P = 128
N_GROUPS = math.ceil(NODES_PER_CORE / P)  # 98
SEG = 4  # groups per segment cell
N_SEGS = math.ceil(N_GROUPS / SEG)  # 25
N_BANKS = 4
BANK = N_NODES // N_BANKS  # 25000
OP_IDX = 2048  # descriptors per gather op
SBATCH = 32  # pairs per streamed S tile
PF_GROUPS = 12  # gather prefetch horizon (groups)
STORE_GROUPS = 8

_compiled_cache = {}


def _plan(cell_len, pair_key):
    """Static structure. cell_len: (N_SEGS, N_BANKS) stream lengths
    (exact max over cores). pair_key: tuple of (g, b, chunk) pairs."""
    cell_len = np.asarray(cell_len)
    # bank stream length: multiple of 16 for the idx-table wrap
    bank_len = [int(-16 * (-cell_len[:, b].sum() // 16))
                for b in range(N_BANKS)]
    cell_off = np.zeros((N_SEGS, N_BANKS), np.int64)
    for b in range(N_BANKS):
        off = 0
        for s in range(N_SEGS):
            cell_off[s, b] = off
            off += int(cell_len[s, b])
    # uniform ops of OP_IDX descs (last partial, 16-granular)
    bank_ops = []
    for b in range(N_BANKS):
        ops = []
        done = 0
        ramp = [512, 1536]  # stagger-killer: all 4 queues engage within ~1.4us
        while done < bank_len[b]:
            want = ramp.pop(0) if ramp else OP_IDX
            n = min(want, bank_len[b] - done)
            ops.append((done, n))
            done += n
        bank_ops.append(ops)
    pairs = [tuple(p) for p in pair_key]
    return {
        "cell_len": cell_len,
        "cell_off": cell_off,
        "bank_len": bank_len,
        "bank_ops": bank_ops,
        "pairs": pairs,
    }


def _build_kernel(key):
    cell_len_key, pair_key = key
    plan = _plan(np.asarray(cell_len_key).reshape(N_SEGS, N_BANKS), pair_key)
    bank_len = plan["bank_len"]
    bank_ops = plan["bank_ops"]
    cell_off = plan["cell_off"]
    cell_len = plan["cell_len"]
    pairs = plan["pairs"]
    n_pairs = len(pairs)
    n_batches = math.ceil(n_pairs / SBATCH)

    nc = bacc.Bacc("TRN2", target_bir_lowering=False, debug=False,
                   num_devices=N_CORES, num_swdge_queues=N_BANKS)
    f16, f32, f8 = mybir.dt.float16, mybir.dt.float32, mybir.dt.float8e4
    x_d = nc.dram_tensor("x", [N_NODES, N_FEAT], f16,
                         kind="ExternalInput").ap()
    idx_ds = [nc.dram_tensor(f"midx{b}", [P, bank_len[b] // 16],
                             mybir.dt.int16, kind="ExternalInput").ap()
              for b in range(N_BANKS)]
    s_d = nc.dram_tensor("msel", [P, n_pairs * P], f8,
                         kind="ExternalInput").ap()
    invd_d = nc.dram_tensor("minvd", [P, N_GROUPS], f32,
                            kind="ExternalInput").ap()
    out_d = nc.dram_tensor("out", [NODES_PER_CORE, N_FEAT], f32,
                           kind="ExternalOutput").ap()

    n_ops_total = sum(len(o) for o in bank_ops)

    with tile.TileContext(nc) as tc, ExitStack() as ctx:
        meta_pool = ctx.enter_context(tc.tile_pool(name="meta", bufs=1))
        idx_pool = ctx.enter_context(
            tc.tile_pool(name="idx", bufs=n_ops_total))
        gat_pool = ctx.enter_context(tc.tile_pool(name="gat", bufs=22))
        sel_pool = ctx.enter_context(tc.tile_pool(name="sel", bufs=10))
        psum_pool = ctx.enter_context(
            tc.tile_pool(name="psum", bufs=8, space="PSUM"))
        out_pool = ctx.enter_context(tc.tile_pool(name="outb", bufs=3))

        # idx tiles up front: first round on GpSimd itself, then the first
        # two S tiles, then the bulk (Sync only; nothing can block these).
        idx_tiles = {}

        def load_idx(b, oi, eng):
            off, n = bank_ops[b][oi]
            i_t = idx_pool.tile([P, OP_IDX // 16], mybir.dt.int16, tag="idx")
            eng.dma_start(out=i_t[:, :n // 16],
                          in_=idx_ds[b][:, off // 16:(off + n) // 16])
            idx_tiles[(b, oi)] = i_t

        # first round on GpSimd itself: same-engine dependency lets the
        # first gathers start ~9 us earlier than a cross-engine sem wait
        for b in range(N_BANKS):
            load_idx(b, 0, nc.gpsimd)

        emitted_ops = [0] * N_BANKS
        op_tiles = {}  # (b, op_i) -> gather tile
        # chunk -> (op index, column within op tile) per bank
        chunk_op = []
        for b in range(N_BANKS):
            m = {}
            for oi, (off, n) in enumerate(bank_ops[b]):
                for j in range(-(-n // 128)):
                    m[off // 128 + j] = (oi, j)
            chunk_op.append(m)

        nreg = {}

        def emit_one_op(b):
            oi = emitted_ops[b]
            off, n = bank_ops[b][oi]
            i_t = idx_tiles[(b, oi)]
            g_t = gat_pool.tile([P, OP_IDX // 128, N_FEAT], f16, tag="gat")
            if n not in nreg:
                nreg[n] = nc.gpsimd.to_reg(n)
            nc.gpsimd.dma_gather(
                out_ap=g_t[:, :-(-n // 128), :],
                in_ap=x_d[b * BANK:(b + 1) * BANK, :],
                idxs_ap=i_t[:, :n // 16],
                num_idxs=n,
                num_idxs_reg=nreg[n],
                elem_size=N_FEAT,
                queue_num=b,
                single_packet=False,
            )
            op_tiles[(b, oi)] = g_t
            emitted_ops[b] += 1

        def emit_until_group(g):
            """Round-robin emission until every bank covers group g's cells."""
            s = min(g // SEG, N_SEGS - 1)
            need_op = [0] * N_BANKS
            for b in range(N_BANKS):
                end = int(cell_off[s, b] + cell_len[s, b])
                if end == 0:
                    need_op[b] = -1
                else:
                    need_op[b] = chunk_op[b][(end - 1) // 128][0]
            progress = True
            while progress:
                progress = False
                for b in range(N_BANKS):
                    if emitted_ops[b] <= need_op[b] and \
                            emitted_ops[b] < len(bank_ops[b]):
                        emit_one_op(b)
                        progress = True

        s_tiles = {}

        def emit_sbatch(bi):
            if bi in s_tiles or bi >= n_batches:
                return
            c0 = bi * SBATCH
            n = min(SBATCH, n_pairs - c0)
            s_t = sel_pool.tile([P, SBATCH * P], f8, tag="sel")
            nc.sync.dma_start(out=s_t[:, :n * P],
                              in_=s_d[:, c0 * P:(c0 + n) * P])
            s_tiles[bi] = s_t

        # group -> list of pair indices (in emission order)
        group_pairs = {}
        for pi, (g, b, c) in enumerate(pairs):
            group_pairs.setdefault(g, []).append(pi)

        # prime the pipeline: first S tiles early on the Sync queue, then
        # the remaining idx tiles, invd, and the gather run-ahead.
        emit_sbatch(0)
        emit_sbatch(1)
        for oi in range(1, max(len(o) for o in bank_ops)):
            for b in range(N_BANKS):
                if oi < len(bank_ops[b]):
                    load_idx(b, oi, nc.sync)
        invd_t = meta_pool.tile([P, N_GROUPS], f32)
        nc.scalar.dma_start(out=invd_t[:], in_=invd_d[:])
        emit_until_group(min(PF_GROUPS, N_GROUPS - 1))

        out_t = None
        for g in range(N_GROUPS):
            if g + PF_GROUPS < N_GROUPS:
                emit_until_group(g + PF_GROUPS)
            plist = group_pairs[g]
            ps = psum_pool.tile([P, N_FEAT], f32)
            for i, pi in enumerate(plist):
                _, b, c = pairs[pi]
                bi = pi // SBATCH
                emit_sbatch(bi)
                emit_sbatch(bi + 1)
                s_t = s_tiles[bi]
                lc = pi - bi * SBATCH
                oi, col = chunk_op[b][c]
                g_t = op_tiles[(b, oi)]
                nc.tensor.matmul(
                    ps[:],
                    lhsT=s_t[:, lc * P:(lc + 1) * P],
                    rhs=g_t[:, col, :],
                    start=(i == 0),
                    stop=(i == len(plist) - 1),
                )
            if g % STORE_GROUPS == 0:
                out_t = out_pool.tile([P, STORE_GROUPS, N_FEAT], f32,
                                      tag="outb")
            nc.scalar.activation(out=out_t[:, g % STORE_GROUPS, :], in_=ps[:],
                                 func=mybir.ActivationFunctionType.Copy,
                                 scale=invd_t[:, g:g + 1])
            if g % STORE_GROUPS == STORE_GROUPS - 1 or g == N_GROUPS - 1:
                g0 = (g // STORE_GROUPS) * STORE_GROUPS
                ngroups = g - g0 + 1
                nfull = ngroups
                rows_last = min(P, NODES_PER_CORE - (g0 + ngroups - 1) * P)
                if rows_last < P:
                    nfull -= 1
                if nfull > 0:
                    dst = out_d[g0 * P:(g0 + nfull) * P, :].rearrange(
                        "(j p) f -> p j f", p=P)
                    nc.scalar.dma_start(out=dst, in_=out_t[:, :nfull, :])
                if nfull < ngroups:
                    gl = g0 + ngroups - 1
                    nc.scalar.dma_start(
                        out=out_d[gl * P:gl * P + rows_last, :],
                        in_=out_t[:rows_last, ngroups - 1, :])
    nc.compile()
    return nc


def _prepare(x, edge_src, edge_dst):
    x16 = np.ascontiguousarray(np.asarray(x), dtype=np.float16)
    src = np.asarray(edge_src).astype(np.int64)
    dst = np.asarray(edge_dst).astype(np.int64)

    deg = np.bincount(dst, minlength=N_NODES)
    inv_deg = (1.0 / np.maximum(deg, 1)).astype(np.float32)

    core_e = dst // NODES_PER_CORE
    ldst = dst % NODES_PER_CORE
    g_e = ldst // P
    s_e = g_e // SEG
    b_e = src // BANK

    cnt = np.zeros((N_CORES, N_SEGS, N_BANKS), np.int64)
    np.add.at(cnt, (core_e, s_e, b_e), 1)
    cell_len = cnt.max(axis=0).astype(np.int64)

    plan = _plan(cell_len, ())
    cell_off = plan["cell_off"]
    bank_len = plan["bank_len"]

    # per-core packing: stream position of every edge
    per_core = []
    pair_set = [set() for _ in range(N_GROUPS)]
    for k in range(N_CORES):
        m = core_e == k
        ksrc, kg, kb, ks = src[m], g_e[m], b_e[m], s_e[m]
        kldst = ldst[m]
        order = np.lexsort((ksrc, kg, kb, ks))
        ksrc, kg, kb, ks, kldst = (ksrc[order], kg[order], kb[order],
                                   ks[order], kldst[order])
        cid = ks * N_BANKS + kb
        pos = np.zeros(len(ksrc), np.int64)
        uniq, starts, counts = np.unique(cid, return_index=True,
                                         return_counts=True)
        for u, st, n in zip(uniq, starts, counts):
            s, b = int(u) // N_BANKS, int(u) % N_BANKS
            assert n <= cell_len[s, b]
            pos[st:st + n] = cell_off[s, b] + np.arange(n)
        chunk = pos // 128
        for g in range(N_GROUPS):
            gm = kg == g
            if not gm.any():
                continue
            for b, c in set(zip(kb[gm].tolist(), chunk[gm].tolist())):
                pair_set[g].add((b, c))
        per_core.append((ksrc, kg, kb, kldst, pos))

    # pair list in emission order (group-major, then bank, then chunk)
    pairs = []
    pair_index = {}
    for g in range(N_GROUPS):
        cells = sorted(pair_set[g])
        if not cells:
            cells = [(0, 0)]  # dummy pair so psum gets written (S is 0)
        for b, c in cells:
            pair_index[(g, b, c)] = len(pairs)
            pairs.append((g, b, c))
    pair_key = tuple(pairs)
    n_pairs = len(pairs)

    in_maps = []
    for k in range(N_CORES):
        ksrc, kg, kb, kldst, pos = per_core[k]
        idxs = {}
        for b in range(N_BANKS):
            st = np.zeros(bank_len[b], np.int16)
            mb = kb == b
            st[pos[mb]] = (ksrc[mb] - b * BANK).astype(np.int16)
            # wrap-16 layout replicated to 128 partitions
            idxs[f"midx{b}"] = np.ascontiguousarray(
                np.tile(st.reshape(-1, 16).T, (8, 1)))
        # host-built S: [slot-in-chunk, pair*128 + dst-in-group] one-hot
        s_tab = np.zeros((P, n_pairs * P), np.float32)
        pidx = np.fromiter(
            (pair_index[(g, b, c)] for g, b, c in
             zip(kg.tolist(), kb.tolist(), (pos // 128).tolist())),
            np.int64, len(kg))
        dd = (kldst - kg * P).astype(np.int64)
        np.add.at(s_tab, (pos % 128, pidx * P + dd), 1.0)
        invd = np.zeros((N_GROUPS * P,), np.float32)
        invd[:NODES_PER_CORE] = inv_deg[k * NODES_PER_CORE:
                                        (k + 1) * NODES_PER_CORE]
        in_maps.append({
            "x": x16,
            **idxs,
            "msel": s_tab.astype(ml_dtypes.float8_e4m3),
            "minvd": np.ascontiguousarray(invd.reshape(N_GROUPS, P).T),
        })
    key = (tuple(int(v) for v in cell_len.ravel()), pair_key)
    kernel.last_stats = {"total_len": int(sum(bank_len)), "n_pairs": n_pairs}
    return in_maps, key


def kernel(x, edge_src, edge_dst, _trace=False):
    in_maps, key = _prepare(x, edge_src, edge_dst)
    nc = _compiled_cache.get(key)
    if nc is None:
        nc = _build_kernel(key)
        _compiled_cache[key] = nc
    res = run_bass_kernel_spmd(nc, in_maps, core_ids=list(range(N_CORES)),
                               trace=_trace)
    out = np.concatenate([res.results[k]["out"] for k in range(N_CORES)],
                         axis=0)
    if _trace:
        kernel.last_exec_time_ns = res.exec_time_ns
        kernel.last_result = res
    return out
